# revision 18
# baseline (speedup 1.0000x reference)
"""Multi-head attention (nn_CustomFlashAttention) for 8 Trainium2 NeuronCores.

Sharding (head/tensor parallel per the problem's hint): each of the 8 cores
owns 2 of the 16 heads. Per core inputs:
  - xT : (2048, 4096) fp32  -- x.T (hidden-major), both batches concatenated
  - wq/wk/wv : (2048, 256)  -- that core's 256 rows of the weight, transposed
  - wo : (256, 2048)        -- that core's 256 columns of w_o, transposed
Each core computes q/k/v projections for its heads, full softmax attention,
and its partial output projection out_i = o_i @ w_o_i.T; the host sums the 8
partial outputs (the mathematical all-reduce of the hint).

All matmuls run in float32r (fp32 storage, reduced-precision multiply) which
streams at 1 cycle/row on the PE for moving dims >= 256.
"""

import numpy as np

# ---- problem constants (hardcoded; kernel.py must be self-contained) ----
B = 2          # batch
S = 2048       # sequence length
D = 2048       # hidden dim
NH = 16        # heads
HD = 128       # head dim
NCORES = 8
HPC = NH // NCORES          # heads per core = 2
E = HPC * HD                # per-core projection width = 256
T = B * S                   # total tokens = 4096
KO = D // 128               # contraction tiles over hidden dim = 16
TBLK = 256                  # token block for the projection phase
SCALE = 1.0 / float(np.sqrt(HD))

_CACHE = {}


def _build_nc():
    import concourse.tile as tile
    from concourse import bacc, mybir

    F32, F32R = mybir.dt.float32, mybir.dt.float32r
    Exp = mybir.ActivationFunctionType.Exp

    nc = bacc.Bacc("TRN2", target_bir_lowering=False)
    xT = nc.dram_tensor("xT", [D, T], F32, kind="ExternalInput")
    wq = nc.dram_tensor("wq", [D, E], F32, kind="ExternalInput")
    wk = nc.dram_tensor("wk", [D, E], F32, kind="ExternalInput")
    wv = nc.dram_tensor("wv", [D, E], F32, kind="ExternalInput")
    wo = nc.dram_tensor("wo", [E, D], F32, kind="ExternalInput")
    out = nc.dram_tensor("out", [T, D], F32, kind="ExternalOutput")

    with tile.TileContext(nc) as tc:
        with (
            tc.tile_pool(name="consts", bufs=1) as consts,
            tc.tile_pool(name="batch", bufs=1) as batch,
            tc.tile_pool(name="xs", bufs=2) as xsp,
            tc.tile_pool(name="pt", bufs=4) as ptp,
            tc.tile_pool(name="small", bufs=4) as small,
            tc.tile_pool(name="recb", bufs=2) as recbp,
            tc.tile_pool(name="outp", bufs=4) as outp,
            tc.tile_pool(name="otp", bufs=2) as otp,
            tc.tile_pool(name="ps", bufs=3, space="PSUM") as ps,
            tc.tile_pool(name="pacc", bufs=2, space="PSUM") as pacc,
            tc.tile_pool(name="psum1", bufs=1, space="PSUM") as psum1,
            tc.tile_pool(name="pj3", bufs=2, space="PSUM") as pj3p,
        ):
            # prefetch the first two x strips before the (larger) weight DMAs
            # so the PE can start as soon as x strip 0 + wq land
            xs_pre = []
            for tb in range(2):
                xs = xsp.tile([128, KO, TBLK], F32R, tag="xs")
                nc.sync.dma_start(
                    xs[:],
                    xT[:, tb * TBLK : (tb + 1) * TBLK].rearrange("(ko p) t -> p ko t", p=128).bitcast(F32R),
                )
                xs_pre.append(xs)
            # persistent weights in SBUF, hidden dim on partitions
            wq_sb = consts.tile([128, KO, E], F32R)
            wk_sb = consts.tile([128, KO, E], F32R)
            wv_sb = consts.tile([128, KO, E], F32R)
            wo_sb = consts.tile([128, HPC, D], F32R)
            # weights ride the scalar-engine DMA queue so x strips on the sync
            # queue aren't stuck behind them
            for ko in range(KO):
                nc.sync.dma_start(wq_sb[:, ko, :], wq[ko * 128 : (ko + 1) * 128, :].bitcast(F32R))
            for ko in range(KO):
                nc.scalar.dma_start(wk_sb[:, ko, :], wk[ko * 128 : (ko + 1) * 128, :].bitcast(F32R))
            for ko in range(KO):
                nc.scalar.dma_start(wv_sb[:, ko, :], wv[ko * 128 : (ko + 1) * 128, :].bitcast(F32R))
            nc.scalar.dma_start(wo_sb[:], wo[:].rearrange("(h p) f -> p h f", p=128).bitcast(F32R))
            ones_f = consts.tile([128, 1], F32)
            nc.vector.memset(ones_f[:], 1.0)
            ones = consts.tile([128, 1], F32R)
            nc.vector.tensor_copy(ones[:], ones_f[:])

            for b in range(B):
                qT = batch.tile([128, HPC, S], F32R, tag="qT")
                kT = batch.tile([128, HPC, S], F32R, tag="kT")
                v = batch.tile([128, S // 128, E], F32R, tag="v")

                # ---- phase 1: q/k/v projections for batch b ----
                for tb in range(S // TBLK):
                    t0 = b * S + tb * TBLK
                    if b == 0 and tb < 2:
                        xs = xs_pre[tb]
                    else:
                        xs = xsp.tile([128, KO, TBLK], F32R, tag="xs")
                        nc.sync.dma_start(
                            xs[:],
                            xT[:, t0 : t0 + TBLK].rearrange("(ko p) t -> p ko t", p=128).bitcast(F32R),
                        )
                    for w_sb, dstT in ((wq_sb, qT), (wk_sb, kT)):
                        for eb in range(HPC):
                            pj = ps.tile([128, TBLK], F32, tag="ps")
                            for ko in range(KO):
                                nc.tensor.matmul(
                                    pj[:],
                                    w_sb[:, ko, eb * 128 : (eb + 1) * 128],
                                    xs[:, ko, :],
                                    start=(ko == 0),
                                    stop=(ko == KO - 1),
                                )
                            nc.vector.tensor_copy(
                                dstT[:, eb, tb * TBLK : tb * TBLK + TBLK], pj[:]
                            )
                    for ts in range(TBLK // 128):
                        pj = ps.tile([128, E], F32, tag="ps")
                        for ko in range(KO):
                            nc.tensor.matmul(
                                pj[:],
                                xs[:, ko, ts * 128 : (ts + 1) * 128],
                                wv_sb[:, ko, :],
                                start=(ko == 0),
                                stop=(ko == KO - 1),
                            )
                        nc.vector.tensor_copy(v[:, (tb * TBLK) // 128 + ts, :], pj[:])

                # ---- phases 2+3 interleaved per query block ----
                # attention for both heads of query block tqb, then the output
                # projection of query block tqb-1 (one-block lag so the
                # normalize chain has slack); out DMAs spread over the span
                NIK = S // 128

                def phase3_block(tqb_o, oT_o):
                    for to in range(tqb_o * 4, (tqb_o + 1) * 4):
                        for fb in range(D // 512):
                            pj = pj3p.tile([128, 512], F32, tag="pj3")
                            for h in range(HPC):
                                nc.tensor.matmul(
                                    pj[:],
                                    oT_o[:, h, (to % 4) * 128 : (to % 4 + 1) * 128],
                                    wo_sb[:, h, fb * 512 : (fb + 1) * 512],
                                    start=(h == 0),
                                    stop=(h == HPC - 1),
                                )
                            ot_sb = outp.tile([128, 512], F32, tag="ot_sb")
                            if (to + fb) % 2 == 0:
                                nc.vector.tensor_copy(ot_sb[:], pj[:])
                            else:
                                nc.scalar.copy(ot_sb[:], pj[:])
                            # alternate store queues to double DMA bandwidth
                            eng = nc.sync if (to + fb) % 2 == 0 else nc.scalar
                            eng.dma_start(
                                out[
                                    b * S + to * 128 : b * S + (to + 1) * 128,
                                    fb * 512 : (fb + 1) * 512,
                                ],
                                ot_sb[:],
                            )

                prev = None  # (tqb, oT tile)
                for tqb in range(S // 512):
                    oT = otp.tile([128, HPC, 512], F32R, tag="oT")
                    for h in range(HPC):
                        po = pacc.tile([128, 512], F32, tag="po")
                        sm = psum1.tile([1, 512], F32, tag="sm")
                        for ik in range(NIK):
                            sc = ps.tile([128, 512], F32, tag="ps")
                            nc.tensor.matmul(
                                sc[:],
                                kT[:, h, ik * 128 : (ik + 1) * 128],
                                qT[:, h, tqb * 512 : (tqb + 1) * 512],
                                start=True,
                                stop=True,
                            )
                            pt = ptp.tile([128, 512], F32R, tag="pt")
                            nc.scalar.activation(pt[:], sc[:], Exp, bias=0.0, scale=SCALE)
                            nc.tensor.matmul(
                                po[:],
                                v[:, ik, h * 128 : (h + 1) * 128],
                                pt[:],
                                start=(ik == 0),
                                stop=(ik == NIK - 1),
                            )
                            nc.tensor.matmul(
                                sm[0:1, :],
                                ones[:],
                                pt[:],
                                start=(ik == 0),
                                stop=(ik == NIK - 1),
                            )
                        rec = small.tile([1, 512], F32, tag="rec")
                        nc.vector.reciprocal(rec[:], sm[0:1, :])
                        recb = recbp.tile([128, 512], F32, tag="recb")
                        nc.gpsimd.partition_broadcast(recb[:], rec[0:1, :])
                        nc.vector.tensor_mul(oT[:, h, :], po[:], recb[:])
                    if prev is not None:
                        phase3_block(*prev)
                    prev = (tqb, oT)
                phase3_block(*prev)
    nc.compile()
    return nc


def _get_nc():
    if "nc" not in _CACHE:
        _CACHE["nc"] = _build_nc()
    return _CACHE["nc"]


def _shard_inputs(x, w_q, w_k, w_v, w_o):
    x = np.asarray(x, dtype=np.float32)
    w_q = np.asarray(w_q, dtype=np.float32)
    w_k = np.asarray(w_k, dtype=np.float32)
    w_v = np.asarray(w_v, dtype=np.float32)
    w_o = np.asarray(w_o, dtype=np.float32)
    xT = np.ascontiguousarray(x.reshape(T, D).T)
    in_maps = []
    for i in range(NCORES):
        e0 = i * E
        in_maps.append(
            {
                "xT": xT,
                "wq": np.ascontiguousarray(w_q[e0 : e0 + E, :].T),
                "wk": np.ascontiguousarray(w_k[e0 : e0 + E, :].T),
                "wv": np.ascontiguousarray(w_v[e0 : e0 + E, :].T),
                "wo": np.ascontiguousarray(w_o[:, e0 : e0 + E].T),
            }
        )
    return in_maps


def run_spmd(x, w_q, w_k, w_v, w_o, **spmd_kwargs):
    """Build+run on cores 0-7; returns (partial results list, BassKernelResults)."""
    from concourse.bass_utils import run_bass_kernel_spmd

    nc = _get_nc()
    in_maps = _shard_inputs(x, w_q, w_k, w_v, w_o)
    res = run_bass_kernel_spmd(nc, in_maps, core_ids=list(range(NCORES)), **spmd_kwargs)
    return res


def kernel(x, w_q, w_k, w_v, w_o):
    res = run_spmd(x, w_q, w_k, w_v, w_o)
    acc = res.results[0]["out"].astype(np.float32)
    for i in range(1, NCORES):
        acc = acc + res.results[i]["out"]
    return acc.reshape(B, S, D)


# revision 24
# speedup vs baseline: 1.0663x; 1.0663x over previous
"""Multi-head attention (nn_CustomFlashAttention) for 8 Trainium2 NeuronCores.

Sharding (head/tensor parallel per the problem's hint): each of the 8 cores
owns 2 of the 16 heads. Per core inputs:
  - xT : (2048, 4096) fp32  -- x.T (hidden-major), both batches concatenated
  - wq/wk/wv : (2048, 256)  -- that core's 256 rows of the weight, transposed
  - wo : (256, 2048)        -- that core's 256 columns of w_o, transposed
Each core computes q/k/v projections for its heads, full softmax attention,
and its partial output projection out_i = o_i @ w_o_i.T; the host sums the 8
partial outputs (the mathematical all-reduce of the hint).

All matmuls run in float32r (fp32 storage, reduced-precision multiply) which
streams at 1 cycle/row on the PE for moving dims >= 256.
"""

import numpy as np

# ---- problem constants (hardcoded; kernel.py must be self-contained) ----
B = 2          # batch
S = 2048       # sequence length
D = 2048       # hidden dim
NH = 16        # heads
HD = 128       # head dim
NCORES = 8
HPC = NH // NCORES          # heads per core = 2
E = HPC * HD                # per-core projection width = 256
T = B * S                   # total tokens = 4096
KO = D // 128               # contraction tiles over hidden dim = 16
TBLK = 256                  # token block for the projection phase
SCALE = 1.0 / float(np.sqrt(HD))

_CACHE = {}


def _build_nc():
    import concourse.tile as tile
    from concourse import bacc, mybir

    F32, F32R = mybir.dt.float32, mybir.dt.float32r
    Exp = mybir.ActivationFunctionType.Exp

    nc = bacc.Bacc("TRN2", target_bir_lowering=False)
    # inputs arrive pre-tiled from the host in the exact SBUF layouts so every
    # DMA is contiguous per partition (16KB packets instead of 1KB)
    NTB = T // TBLK
    xB = nc.dram_tensor("xB", [NTB, 128, KO, TBLK], F32, kind="ExternalInput")
    wq = nc.dram_tensor("wq", [128, KO, E], F32, kind="ExternalInput")
    wk = nc.dram_tensor("wk", [128, KO, E], F32, kind="ExternalInput")
    wv = nc.dram_tensor("wv", [128, KO, E], F32, kind="ExternalInput")
    wo = nc.dram_tensor("wo", [128, HPC, D], F32, kind="ExternalInput")
    out = nc.dram_tensor("out", [T, D], F32, kind="ExternalOutput")

    with tile.TileContext(nc) as tc:
        with (
            tc.tile_pool(name="consts", bufs=1) as consts,
            tc.tile_pool(name="batch", bufs=1) as batch,
            tc.tile_pool(name="xs", bufs=2) as xsp,
            tc.tile_pool(name="pt", bufs=4) as ptp,
            tc.tile_pool(name="small", bufs=4) as small,
            tc.tile_pool(name="recb", bufs=2) as recbp,
            tc.tile_pool(name="outp", bufs=2) as outp,
            tc.tile_pool(name="otp", bufs=2) as otp,
            tc.tile_pool(name="ps", bufs=3, space="PSUM") as ps,
            tc.tile_pool(name="pacc", bufs=2, space="PSUM") as pacc,
            tc.tile_pool(name="psum1", bufs=1, space="PSUM") as psum1,
            tc.tile_pool(name="pj3", bufs=2, space="PSUM") as pj3p,
        ):
            # prefetch the first two x strips before the (larger) weight DMAs
            # so the PE can start as soon as x strip 0 + wq land
            xs_pre = []
            for tb in range(2):
                xs = xsp.tile([128, KO, TBLK], F32R, tag="xs")
                nc.sync.dma_start(xs[:], xB[tb].bitcast(F32R))
                xs_pre.append(xs)
            # persistent weights in SBUF, hidden dim on partitions; wq on the
            # sync queue (needed first), the rest on the scalar-engine queue
            wq_sb = consts.tile([128, KO, E], F32R)
            wk_sb = consts.tile([128, KO, E], F32R)
            wv_sb = consts.tile([128, KO, E], F32R)
            wo_sb = consts.tile([128, HPC, D], F32R)
            nc.sync.dma_start(wq_sb[:], wq[:].bitcast(F32R))
            nc.scalar.dma_start(wk_sb[:], wk[:].bitcast(F32R))
            nc.scalar.dma_start(wv_sb[:], wv[:].bitcast(F32R))
            nc.scalar.dma_start(wo_sb[:], wo[:].bitcast(F32R))
            ones_f = consts.tile([128, 1], F32)
            nc.vector.memset(ones_f[:], 1.0)
            ones = consts.tile([128, 1], F32R)
            nc.vector.tensor_copy(ones[:], ones_f[:])

            for b in range(B):
                qT = batch.tile([128, HPC, S], F32R, tag="qT")
                kT = batch.tile([128, HPC, S], F32R, tag="kT")
                v = batch.tile([128, S // 128, E], F32R, tag="v")

                # ---- phase 1: q/k/v projections for batch b ----
                for tb in range(S // TBLK):
                    tbid = b * (S // TBLK) + tb
                    if tbid < 2:
                        xs = xs_pre[tbid]
                    else:
                        xs = xsp.tile([128, KO, TBLK], F32R, tag="xs")
                        nc.sync.dma_start(xs[:], xB[tbid].bitcast(F32R))
                    for w_sb, dstT in ((wq_sb, qT), (wk_sb, kT)):
                        for eb in range(HPC):
                            pj = ps.tile([128, TBLK], F32, tag="ps")
                            for ko in range(KO):
                                nc.tensor.matmul(
                                    pj[:],
                                    w_sb[:, ko, eb * 128 : (eb + 1) * 128],
                                    xs[:, ko, :],
                                    start=(ko == 0),
                                    stop=(ko == KO - 1),
                                )
                            nc.vector.tensor_copy(
                                dstT[:, eb, tb * TBLK : tb * TBLK + TBLK], pj[:]
                            )
                    for ts in range(TBLK // 128):
                        pj = ps.tile([128, E], F32, tag="ps")
                        for ko in range(KO):
                            nc.tensor.matmul(
                                pj[:],
                                xs[:, ko, ts * 128 : (ts + 1) * 128],
                                wv_sb[:, ko, :],
                                start=(ko == 0),
                                stop=(ko == KO - 1),
                            )
                        nc.vector.tensor_copy(v[:, (tb * TBLK) // 128 + ts, :], pj[:])

                # ---- phases 2+3 interleaved per query block ----
                # attention for both heads of query block tqb, then the output
                # projection of query block tqb-1 (one-block lag so the
                # normalize chain has slack); out DMAs spread over the span
                NIK = S // 128

                def phase3_block(tqb_o, oT_o):
                    for to in range(tqb_o * 4, (tqb_o + 1) * 4):
                        # full 2048-wide row block -> one contiguous 8KB/partition store
                        ot_sb = outp.tile([128, D], F32, tag="ot_sb")
                        for fb in range(D // 512):
                            pj = pj3p.tile([128, 512], F32, tag="pj3")
                            for h in range(HPC):
                                nc.tensor.matmul(
                                    pj[:],
                                    oT_o[:, h, (to % 4) * 128 : (to % 4 + 1) * 128],
                                    wo_sb[:, h, fb * 512 : (fb + 1) * 512],
                                    start=(h == 0),
                                    stop=(h == HPC - 1),
                                )
                            if fb % 2 == 0:
                                nc.vector.tensor_copy(ot_sb[:, fb * 512 : (fb + 1) * 512], pj[:])
                            else:
                                nc.scalar.copy(ot_sb[:, fb * 512 : (fb + 1) * 512], pj[:])
                        eng = nc.sync if to % 2 == 0 else nc.scalar
                        eng.dma_start(
                            out[b * S + to * 128 : b * S + (to + 1) * 128, :],
                            ot_sb[:],
                        )

                prev = None  # (tqb, oT tile)
                for tqb in range(S // 512):
                    oT = otp.tile([128, HPC, 512], F32R, tag="oT")
                    for h in range(HPC):
                        po = pacc.tile([128, 512], F32, tag="po")
                        sm = psum1.tile([1, 512], F32, tag="sm")
                        for ik in range(NIK):
                            sc = ps.tile([128, 512], F32, tag="ps")
                            nc.tensor.matmul(
                                sc[:],
                                kT[:, h, ik * 128 : (ik + 1) * 128],
                                qT[:, h, tqb * 512 : (tqb + 1) * 512],
                                start=True,
                                stop=True,
                            )
                            pt = ptp.tile([128, 512], F32R, tag="pt")
                            nc.scalar.activation(pt[:], sc[:], Exp, bias=0.0, scale=SCALE)
                            nc.tensor.matmul(
                                po[:],
                                v[:, ik, h * 128 : (h + 1) * 128],
                                pt[:],
                                start=(ik == 0),
                                stop=(ik == NIK - 1),
                            )
                            nc.tensor.matmul(
                                sm[0:1, :],
                                ones[:],
                                pt[:],
                                start=(ik == 0),
                                stop=(ik == NIK - 1),
                            )
                        rec = small.tile([1, 512], F32, tag="rec")
                        nc.vector.reciprocal(rec[:], sm[0:1, :])
                        recb = recbp.tile([128, 512], F32, tag="recb")
                        nc.gpsimd.partition_broadcast(recb[:], rec[0:1, :])
                        nc.vector.tensor_mul(oT[:, h, :], po[:], recb[:])
                    if prev is not None:
                        phase3_block(*prev)
                    prev = (tqb, oT)
                phase3_block(*prev)
    nc.compile()
    return nc


def _get_nc():
    if "nc" not in _CACHE:
        _CACHE["nc"] = _build_nc()
    return _CACHE["nc"]


def _shard_inputs(x, w_q, w_k, w_v, w_o):
    x = np.asarray(x, dtype=np.float32)
    w_q = np.asarray(w_q, dtype=np.float32)
    w_k = np.asarray(w_k, dtype=np.float32)
    w_v = np.asarray(w_v, dtype=np.float32)
    w_o = np.asarray(w_o, dtype=np.float32)
    # x pre-tiled to [tb, p, ko, t] so each x-strip DMA is fully contiguous
    xB = np.ascontiguousarray(
        x.reshape(T // TBLK, TBLK, KO, 128).transpose(0, 3, 2, 1)
    )

    def wqkv_tile(w, e0):
        # [128, KO, E]: (p, ko, e) = w[e0+e, ko*128+p]
        return np.ascontiguousarray(
            w[e0 : e0 + E, :].reshape(E, KO, 128).transpose(2, 1, 0)
        )

    in_maps = []
    for i in range(NCORES):
        e0 = i * E
        # wo [128, HPC, D]: (p, h, f) = w_o[f, e0 + h*128 + p]
        wo_t = np.ascontiguousarray(
            w_o[:, e0 : e0 + E].reshape(D, HPC, 128).transpose(2, 1, 0)
        )
        in_maps.append(
            {
                "xB": xB,
                "wq": wqkv_tile(w_q, e0),
                "wk": wqkv_tile(w_k, e0),
                "wv": wqkv_tile(w_v, e0),
                "wo": wo_t,
            }
        )
    return in_maps


def run_spmd(x, w_q, w_k, w_v, w_o, **spmd_kwargs):
    """Build+run on cores 0-7; returns (partial results list, BassKernelResults)."""
    from concourse.bass_utils import run_bass_kernel_spmd

    nc = _get_nc()
    in_maps = _shard_inputs(x, w_q, w_k, w_v, w_o)
    res = run_bass_kernel_spmd(nc, in_maps, core_ids=list(range(NCORES)), **spmd_kwargs)
    return res


def kernel(x, w_q, w_k, w_v, w_o):
    res = run_spmd(x, w_q, w_k, w_v, w_o)
    acc = res.results[0]["out"].astype(np.float32)
    for i in range(1, NCORES):
        acc = acc + res.results[i]["out"]
    return acc.reshape(B, S, D)


# revision 25
# speedup vs baseline: 1.0906x; 1.0228x over previous
"""Multi-head attention (nn_CustomFlashAttention) for 8 Trainium2 NeuronCores.

Sharding (head/tensor parallel per the problem's hint): each of the 8 cores
owns 2 of the 16 heads. Per core inputs:
  - xT : (2048, 4096) fp32  -- x.T (hidden-major), both batches concatenated
  - wq/wk/wv : (2048, 256)  -- that core's 256 rows of the weight, transposed
  - wo : (256, 2048)        -- that core's 256 columns of w_o, transposed
Each core computes q/k/v projections for its heads, full softmax attention,
and its partial output projection out_i = o_i @ w_o_i.T; the host sums the 8
partial outputs (the mathematical all-reduce of the hint).

All matmuls run in float32r (fp32 storage, reduced-precision multiply) which
streams at 1 cycle/row on the PE for moving dims >= 256.
"""

import numpy as np

# ---- problem constants (hardcoded; kernel.py must be self-contained) ----
B = 2          # batch
S = 2048       # sequence length
D = 2048       # hidden dim
NH = 16        # heads
HD = 128       # head dim
NCORES = 8
HPC = NH // NCORES          # heads per core = 2
E = HPC * HD                # per-core projection width = 256
T = B * S                   # total tokens = 4096
KO = D // 128               # contraction tiles over hidden dim = 16
TBLK = 256                  # token block for the projection phase
SCALE = 1.0 / float(np.sqrt(HD))

_CACHE = {}


def _build_nc():
    import concourse.tile as tile
    from concourse import bacc, mybir

    F32, F32R = mybir.dt.float32, mybir.dt.float32r
    Exp = mybir.ActivationFunctionType.Exp

    nc = bacc.Bacc("TRN2", target_bir_lowering=False)
    # inputs arrive pre-tiled from the host in the exact SBUF layouts so every
    # DMA is contiguous per partition (16KB packets instead of 1KB)
    NTB = T // TBLK
    xB = nc.dram_tensor("xB", [NTB, 128, KO, TBLK], F32, kind="ExternalInput")
    wq = nc.dram_tensor("wq", [128, KO, E], F32, kind="ExternalInput")
    wk = nc.dram_tensor("wk", [128, KO, E], F32, kind="ExternalInput")
    wv = nc.dram_tensor("wv", [128, KO, E], F32, kind="ExternalInput")
    wo = nc.dram_tensor("wo", [128, HPC, D], F32, kind="ExternalInput")
    out = nc.dram_tensor("out", [T, D], F32, kind="ExternalOutput")

    with tile.TileContext(nc) as tc:
        with (
            tc.tile_pool(name="consts", bufs=1) as consts,
            tc.tile_pool(name="batch", bufs=1) as batch,
            tc.tile_pool(name="xs", bufs=2) as xsp,
            tc.tile_pool(name="pt", bufs=4) as ptp,
            tc.tile_pool(name="small", bufs=4) as small,
            tc.tile_pool(name="recb", bufs=2) as recbp,
            tc.tile_pool(name="outp", bufs=2) as outp,
            tc.tile_pool(name="otp", bufs=2) as otp,
            tc.tile_pool(name="ps", bufs=2, space="PSUM") as ps,
            tc.tile_pool(name="pacc", bufs=2, space="PSUM") as pacc,
            tc.tile_pool(name="psum1", bufs=2, space="PSUM") as psum1,
            tc.tile_pool(name="pj3", bufs=2, space="PSUM") as pj3p,
        ):
            # prefetch the first two x strips before the (larger) weight DMAs
            # so the PE can start as soon as x strip 0 + wq land
            # wq first on the sync queue (gates the very first matmuls),
            # then the first two x strips; remaining weights on the scalar queue
            wq_sb = consts.tile([128, KO, E], F32R)
            wk_sb = consts.tile([128, KO, E], F32R)
            wv_sb = consts.tile([128, KO, E], F32R)
            wo_sb = consts.tile([128, HPC, D], F32R)
            nc.sync.dma_start(wq_sb[:], wq[:].bitcast(F32R))
            xs_pre = []
            for tb in range(2):
                xs = xsp.tile([128, KO, TBLK], F32R, tag="xs")
                nc.sync.dma_start(xs[:], xB[tb].bitcast(F32R))
                xs_pre.append(xs)
            nc.scalar.dma_start(wk_sb[:], wk[:].bitcast(F32R))
            nc.scalar.dma_start(wv_sb[:], wv[:].bitcast(F32R))
            nc.scalar.dma_start(wo_sb[:], wo[:].bitcast(F32R))
            ones_f = consts.tile([128, 1], F32)
            nc.vector.memset(ones_f[:], 1.0)
            ones = consts.tile([128, 1], F32R)
            nc.vector.tensor_copy(ones[:], ones_f[:])

            for b in range(B):
                qT = batch.tile([128, HPC, S], F32R, tag="qT")
                kT = batch.tile([128, HPC, S], F32R, tag="kT")
                v = batch.tile([128, S // 128, E], F32R, tag="v")

                # ---- phase 1: q/k/v projections for batch b ----
                for tb in range(S // TBLK):
                    tbid = b * (S // TBLK) + tb
                    if tbid < 2:
                        xs = xs_pre[tbid]
                    else:
                        xs = xsp.tile([128, KO, TBLK], F32R, tag="xs")
                        nc.sync.dma_start(xs[:], xB[tbid].bitcast(F32R))
                    for w_sb, dstT in ((wq_sb, qT), (wk_sb, kT)):
                        for eb in range(HPC):
                            pj = ps.tile([128, TBLK], F32, tag="ps")
                            for ko in range(KO):
                                nc.tensor.matmul(
                                    pj[:],
                                    w_sb[:, ko, eb * 128 : (eb + 1) * 128],
                                    xs[:, ko, :],
                                    start=(ko == 0),
                                    stop=(ko == KO - 1),
                                )
                            nc.vector.tensor_copy(
                                dstT[:, eb, tb * TBLK : tb * TBLK + TBLK], pj[:]
                            )
                    for ts in range(TBLK // 128):
                        pj = ps.tile([128, E], F32, tag="ps")
                        for ko in range(KO):
                            nc.tensor.matmul(
                                pj[:],
                                xs[:, ko, ts * 128 : (ts + 1) * 128],
                                wv_sb[:, ko, :],
                                start=(ko == 0),
                                stop=(ko == KO - 1),
                            )
                        nc.vector.tensor_copy(v[:, (tb * TBLK) // 128 + ts, :], pj[:])

                # ---- phases 2+3 interleaved per query block ----
                # attention for both heads of query block tqb, then the output
                # projection of query block tqb-1 (one-block lag so the
                # normalize chain has slack); out DMAs spread over the span
                NIK = S // 128

                def phase3_block(tqb_o, oT_o):
                    for to in range(tqb_o * 4, (tqb_o + 1) * 4):
                        # full 2048-wide row block -> one contiguous 8KB/partition store
                        ot_sb = outp.tile([128, D], F32, tag="ot_sb")
                        for fb in range(D // 512):
                            pj = pj3p.tile([128, 512], F32, tag="pj3")
                            for h in range(HPC):
                                nc.tensor.matmul(
                                    pj[:],
                                    oT_o[:, h, (to % 4) * 128 : (to % 4 + 1) * 128],
                                    wo_sb[:, h, fb * 512 : (fb + 1) * 512],
                                    start=(h == 0),
                                    stop=(h == HPC - 1),
                                )
                            if fb % 2 == 0:
                                nc.vector.tensor_copy(ot_sb[:, fb * 512 : (fb + 1) * 512], pj[:])
                            else:
                                nc.scalar.copy(ot_sb[:, fb * 512 : (fb + 1) * 512], pj[:])
                        eng = nc.sync if to % 2 == 0 else nc.scalar
                        eng.dma_start(
                            out[b * S + to * 128 : b * S + (to + 1) * 128, :],
                            ot_sb[:],
                        )

                prev = None  # (tqb, oT tile)
                for tqb in range(S // 512):
                    oT = otp.tile([128, HPC, 512], F32R, tag="oT")
                    for h in range(HPC):
                        po = pacc.tile([128, 512], F32, tag="po")
                        sm = psum1.tile([1, 512], F32, tag="sm")
                        for ik in range(NIK):
                            sc = ps.tile([128, 512], F32, tag="ps")
                            nc.tensor.matmul(
                                sc[:],
                                kT[:, h, ik * 128 : (ik + 1) * 128],
                                qT[:, h, tqb * 512 : (tqb + 1) * 512],
                                start=True,
                                stop=True,
                            )
                            pt = ptp.tile([128, 512], F32R, tag="pt")
                            nc.scalar.activation(pt[:], sc[:], Exp, bias=0.0, scale=SCALE)
                            nc.tensor.matmul(
                                po[:],
                                v[:, ik, h * 128 : (h + 1) * 128],
                                pt[:],
                                start=(ik == 0),
                                stop=(ik == NIK - 1),
                            )
                            nc.tensor.matmul(
                                sm[0:1, :],
                                ones[:],
                                pt[:],
                                start=(ik == 0),
                                stop=(ik == NIK - 1),
                            )
                        rec = small.tile([1, 512], F32, tag="rec")
                        nc.vector.reciprocal(rec[:], sm[0:1, :])
                        recb = recbp.tile([128, 512], F32, tag="recb")
                        nc.gpsimd.partition_broadcast(recb[:], rec[0:1, :])
                        nc.vector.tensor_mul(oT[:, h, :], po[:], recb[:])
                    if prev is not None:
                        phase3_block(*prev)
                    prev = (tqb, oT)
                phase3_block(*prev)
    nc.compile()
    return nc


def _get_nc():
    if "nc" not in _CACHE:
        _CACHE["nc"] = _build_nc()
    return _CACHE["nc"]


def _shard_inputs(x, w_q, w_k, w_v, w_o):
    x = np.asarray(x, dtype=np.float32)
    w_q = np.asarray(w_q, dtype=np.float32)
    w_k = np.asarray(w_k, dtype=np.float32)
    w_v = np.asarray(w_v, dtype=np.float32)
    w_o = np.asarray(w_o, dtype=np.float32)
    # x pre-tiled to [tb, p, ko, t] so each x-strip DMA is fully contiguous
    xB = np.ascontiguousarray(
        x.reshape(T // TBLK, TBLK, KO, 128).transpose(0, 3, 2, 1)
    )

    def wqkv_tile(w, e0):
        # [128, KO, E]: (p, ko, e) = w[e0+e, ko*128+p]
        return np.ascontiguousarray(
            w[e0 : e0 + E, :].reshape(E, KO, 128).transpose(2, 1, 0)
        )

    in_maps = []
    for i in range(NCORES):
        e0 = i * E
        # wo [128, HPC, D]: (p, h, f) = w_o[f, e0 + h*128 + p]
        wo_t = np.ascontiguousarray(
            w_o[:, e0 : e0 + E].reshape(D, HPC, 128).transpose(2, 1, 0)
        )
        in_maps.append(
            {
                "xB": xB,
                "wq": wqkv_tile(w_q, e0),
                "wk": wqkv_tile(w_k, e0),
                "wv": wqkv_tile(w_v, e0),
                "wo": wo_t,
            }
        )
    return in_maps


def run_spmd(x, w_q, w_k, w_v, w_o, **spmd_kwargs):
    """Build+run on cores 0-7; returns (partial results list, BassKernelResults)."""
    from concourse.bass_utils import run_bass_kernel_spmd

    nc = _get_nc()
    in_maps = _shard_inputs(x, w_q, w_k, w_v, w_o)
    res = run_bass_kernel_spmd(nc, in_maps, core_ids=list(range(NCORES)), **spmd_kwargs)
    return res


def kernel(x, w_q, w_k, w_v, w_o):
    res = run_spmd(x, w_q, w_k, w_v, w_o)
    acc = res.results[0]["out"].astype(np.float32)
    for i in range(1, NCORES):
        acc = acc + res.results[i]["out"]
    return acc.reshape(B, S, D)
